# revision 1
# baseline (speedup 1.0000x reference)
"""MLA (multi-head latent attention) Bass kernel for Trainium2, 8 NeuronCores.

Sharding: pure data-parallel over batch (B=8 -> one batch element per core).
Each core runs the full per-batch computation; no collectives.

Layouts (per core):
  - Activations are kept feature-major ("fm"): [feature_partitions, tokens],
    so every projection Y = X @ W becomes  Y_fm = W.T @ X_fm with W stored in
    its natural [in_feat, out_feat] orientation as the matmul lhsT.
  - v_c is computed token-major directly (lhsT = c_kv_fm) so attn@v needs no
    transpose.
  - Scores are computed transposed (scores_T[k_pos, q_pos]) so that
      E_T = exp(scores_T)  serves directly as the rhs of attn@v, and row sums
    come from a ones-vector matmul.
  - Softmax skips max-subtraction (scores are small: |s/scale| < ~3), exp is
    fused with the 1/scale into one ScalarE activation.
  - RoPE: the interleaved (even,odd) pairs are de-interleaved on the host by
    permuting Wqr/Wkr columns, so on-chip rotation is 6 tensor_tensor ops per
    256-row half-block. Dot products are invariant to the permutation since
    it is applied to both q_r and k_r.
  - Normalization (divide by softmax sum) is deferred: o_raw accumulates
    unnormalized, reciprocals of all 16 heads' sums are computed in one DVE
    op, broadcast across partitions with a K=1 matmul, and fused with the
    +b_uv bias (valid because sum_k attn = 1).

All matmul inputs are bf16 (fp32 accumulate in PSUM); final output fp32.
"""

import sys

import numpy as np
import ml_dtypes

for _p in ("/opt/trn_rl_repo",):
    if _p not in sys.path:
        sys.path.append(_p)

B, S, D, H = 8, 512, 2048, 16
DOWN, UP, R = 512, 2048, 512
VHD = UP // H          # 128
HD = D // H            # 128
SCALE = float(HD**0.5 + R**0.5)
P = 128
BF16 = ml_dtypes.bfloat16

_CACHE = {}


def _rope_tables_np():
    pos = np.arange(R, dtype=np.float32)
    div = np.exp(np.arange(0, R, 2, dtype=np.float32) * (-np.log(10000.0) / R))
    theta = np.outer(pos, div)          # [512, 256]
    return np.sin(theta), np.cos(theta)


def build_nc():
    """Build + compile the per-core Bass program. Returns (nc, input_names)."""
    import concourse.mybir as mybir
    import concourse.tile as tile
    from concourse import bacc

    f32 = mybir.dt.float32
    bf16 = mybir.dt.bfloat16
    Ident = mybir.ActivationFunctionType.Identity
    Exp = mybir.ActivationFunctionType.Exp
    MUL = mybir.AluOpType.mult
    ADD = mybir.AluOpType.add
    SUB = mybir.AluOpType.subtract

    nc = bacc.Bacc(
        "TRN2",
        target_bir_lowering=False,
        debug=False,
        enable_asserts=False,
        num_devices=8,
    )

    def din(name, shape, dt=bf16):
        return nc.dram_tensor(name, list(shape), dt, kind="ExternalInput").ap()

    # X^T and stage-1 weights in 4 chunks of 4 k-tiles each (earlier PE start).
    xt_d = [din(f"xt{c}", (P, 4, S)) for c in range(4)]
    wdq_d = [din(f"wdq{c}", (P, 4, DOWN)) for c in range(4)]
    wdkv_d = [din(f"wdkv{c}", (P, 4, DOWN)) for c in range(4)]
    wkr_d = [din(f"wkr{c}", (P, 4, R)) for c in range(4)]
    wuq_d = din("wuq", (H, P, 4, VHD))
    wuk_d = din("wuk", (H, P, 4, VHD))
    wuv_d = din("wuv", (P, 4, UP))           # [p, kt, head-group-major feats]
    wqr_d = din("wqr", (H, P, 4, R))
    wfc_d = din("wfc", (16, P, 16, P))
    cos_d = din("cosr", (P, 2, S))
    sin_d = din("sinr", (P, 2, S))
    bdq_d = din("bdq", (P, 4), f32)
    bdkv_d = din("bdkv", (P, 4), f32)
    bkr_d = din("bkr", (P, 4), f32)
    buq_d = din("buq", (P, H), f32)
    buk_d = din("buk", (P, H), f32)
    buv_d = din("buv", (P, H), f32)
    bqr_d = din("bqr", (P, 64), f32)
    bfc_d = din("bfc", (P, 16), f32)
    yt_d = nc.dram_tensor("yt", [D, S], f32, kind="ExternalOutput").ap()

    input_names = (
        [f"xt{c}" for c in range(4)]
        + [f"wdq{c}" for c in range(4)]
        + [f"wdkv{c}" for c in range(4)]
        + [f"wkr{c}" for c in range(4)]
        + ["wuq", "wuk", "wuv", "wqr", "wfc", "cosr", "sinr",
           "bdq", "bdkv", "bkr", "buq", "buk", "buv", "bqr", "bfc"]
    )

    with tile.TileContext(nc) as tc:
        with (
            tc.tile_pool(name="pconst", bufs=1) as pconst,
            tc.tile_pool(name="pbig", bufs=1) as pbig,
            tc.tile_pool(name="pwork", bufs=2) as pwork,
            tc.tile_pool(name="pps", bufs=7, space="PSUM") as pps,
            tc.tile_pool(name="pps1", bufs=1, space="PSUM") as pps1,
            tc.tile_pool(name="pdram", bufs=1, space="DRAM") as pdram,
        ):
            # ---- stage-0 DMAs, ordered so the first matmuls start ASAP ----
            XT, WDQ, WDKV, WKR = [], [], [], []

            def load_chunk(lst, dram, nm, split=False):
                t = pbig.tile([P, 4, 512], bf16, tag=nm)
                if split:  # finer granularity so the first matmul starts ASAP
                    nc.sync.dma_start(t[:, 0:1, :], dram[:, 0:1, :])
                    nc.sync.dma_start(t[:, 1:4, :], dram[:, 1:4, :])
                else:
                    nc.sync.dma_start(t[:], dram[:])
                lst.append(t)

            load_chunk(XT, xt_d[0], "xt_0", split=True)
            load_chunk(WDQ, wdq_d[0], "w1_0", split=True)
            for c in range(1, 4):
                load_chunk(XT, xt_d[c], f"xt_{c}")
                load_chunk(WDQ, wdq_d[c], f"w1_{c}")
            bias = {}

            def load_bias(nm, ap_, k):
                t = pconst.tile([P, k], f32, tag=f"b_{nm}")
                nc.sync.dma_start(t[:], ap_[:])
                bias[nm] = t

            load_bias("bdq", bdq_d, 4)
            for c in range(4):
                load_chunk(WDKV, wdkv_d[c], f"w1_{4 + c}")
            load_bias("bdkv", bdkv_d, 4)
            for c in range(4):
                load_chunk(WKR, wkr_d[c], f"w1_{8 + c}")
            load_bias("bkr", bkr_d, 4)
            cos_t = pconst.tile([P, 2, S], bf16, tag="cos_t")
            sin_t = pconst.tile([P, 2, S], bf16, tag="sin_t")
            nc.sync.dma_start(cos_t[:], cos_d[:])
            nc.sync.dma_start(sin_t[:], sin_d[:])
            WUV = pbig.tile([P, 4, UP], bf16, tag="wuv")
            nc.sync.dma_start(WUV[:], wuv_d[:])
            for nm, ap_, k in (
                ("buq", buq_d, H), ("buk", buk_d, H), ("buv", buv_d, H),
                ("bqr", bqr_d, 64), ("bfc", bfc_d, 16),
            ):
                load_bias(nm, ap_, k)
            ones_col = pconst.tile([P, 1], bf16, tag="ones_col")
            nc.vector.memset(ones_col[:], 1.0)

            # persistent activations
            CQ = pbig.tile([P, 4, S], bf16, tag="cq")
            CKV = pbig.tile([P, 4, S], bf16, tag="ckv")
            KROT = pbig.tile([P, 4, S], bf16, tag="krot")
            VC = pbig.tile([P, 4, UP], bf16, tag="vc")
            ORAW = pbig.tile([P, H, S], bf16, tag="oraw")
            sums_dram = pdram.tile([H * S], f32)
            recip_dram = pdram.tile([H * S], bf16)

            def rope(dst, src, tmp_prefix):
                # src/dst: [P, 4, S] bf16; halves: tiles 0-1 = x1, 2-3 = x2
                for i in range(2):
                    x1 = src[:, i, :]
                    x2 = src[:, 2 + i, :]
                    c_ = cos_t[:, i, :]
                    s_ = sin_t[:, i, :]
                    t1 = pwork.tile([P, S], bf16, tag=f"{tmp_prefix}a")
                    t2 = pwork.tile([P, S], bf16, tag=f"{tmp_prefix}b")
                    nc.vector.tensor_tensor(t1[:], x1, c_, MUL)
                    nc.vector.tensor_tensor(t2[:], x2, s_, MUL)
                    nc.vector.tensor_tensor(dst[:, i, :], t1[:], t2[:], SUB)
                    t3 = pwork.tile([P, S], bf16, tag=f"{tmp_prefix}a")
                    t4 = pwork.tile([P, S], bf16, tag=f"{tmp_prefix}b")
                    nc.vector.tensor_tensor(t3[:], x2, c_, MUL)
                    nc.vector.tensor_tensor(t4[:], x1, s_, MUL)
                    nc.vector.tensor_tensor(dst[:, 2 + i, :], t3[:], t4[:], ADD)

            # ---- stage 1: c_q, c_kv, k_r(+rope) ----
            # kt-outer so matmuls start as soon as chunk 0 lands and stream
            # with the remaining chunk DMAs (4 psum accumulators at a time)
            KRAW = pbig.tile([P, 4, S], bf16, tag="kraw")
            for dst, W, b in (
                (CQ, WDQ, "bdq"), (CKV, WDKV, "bdkv"), (KRAW, WKR, "bkr"),
            ):
                pss = [
                    pps.tile([P, 512], f32, tag="ps", name=f"ps_s1_{i}")
                    for i in range(4)
                ]
                for kt in range(16):
                    for mt in range(4):
                        nc.tensor.matmul(
                            pss[mt][:],
                            W[kt // 4][:, kt % 4, mt * P:(mt + 1) * P],
                            XT[kt // 4][:, kt % 4, :],
                            start=(kt == 0),
                            stop=(kt == 15),
                        )
                for mt in range(4):
                    nc.scalar.activation(
                        dst[:, mt, :], pss[mt][:], Ident,
                        bias=bias[b][:, mt:mt + 1],
                    )
            rope(KROT, KRAW, "kr")

            # ---- stage 2: v_c token-major (no bias; folded into o-norm) ----
            for tt in range(4):
                for hg in range(4):
                    ps = pps.tile([P, 512], f32, tag="ps")
                    for kt in range(4):
                        nc.tensor.matmul(
                            ps[:],
                            CKV[:, kt, tt * P:(tt + 1) * P],
                            WUV[:, kt, hg * 512:(hg + 1) * 512],
                            start=(kt == 0),
                            stop=(kt == 3),
                        )
                    nc.any.tensor_copy(VC[:, tt, hg * 512:(hg + 1) * 512], ps[:])

            # ---- stage 3: per-head attention, software-pipelined ----
            # Phase A(h): weight DMAs + projections (q_r raw, rope, q_c, k_c)
            # Phase B(h): scores/exp, attn@v, sums
            # Emitted as A(0), A(1), B(0), A(2), B(1), ... so the PE stream of
            # A(h+1) covers B(h)'s wait on rope(h) (the engine streams are
            # statically ordered by the scheduler).
            qk_tiles = {}

            def phase_a(h):
                WQRh = pwork.tile([P, 4, R], bf16, tag="wqrh")
                nc.sync.dma_start(WQRh[:], wqr_d[h])
                WUQh = pwork.tile([P, 4, VHD], bf16, tag="wuqh")
                nc.sync.dma_start(WUQh[:], wuq_d[h])
                WUKh = pwork.tile([P, 4, VHD], bf16, tag="wukh")
                nc.sync.dma_start(WUKh[:], wuk_d[h])

                # q_r raw projection [R, S] fm
                QRAW = pwork.tile([P, 4, S], bf16, tag="qraw")
                for mt in range(4):
                    ps = pps.tile([P, 512], f32, tag="ps")
                    for kt in range(4):
                        nc.tensor.matmul(
                            ps[:],
                            WQRh[:, kt, mt * P:(mt + 1) * P],
                            CQ[:, kt, :],
                            start=(kt == 0),
                            stop=(kt == 3),
                        )
                    nc.scalar.activation(
                        QRAW[:, mt, :], ps[:], Ident,
                        bias=bias["bqr"][:, h * 4 + mt:h * 4 + mt + 1],
                    )
                # q_c, k_c [VHD, S] fm
                qc = pwork.tile([P, S], bf16, tag="qc")
                kc = pwork.tile([P, S], bf16, tag="kc")
                for dst, Wh, b, src in (
                    (qc, WUQh, "buq", CQ), (kc, WUKh, "buk", CKV),
                ):
                    ps = pps.tile([P, 512], f32, tag="ps")
                    for kt in range(4):
                        nc.tensor.matmul(
                            ps[:], Wh[:, kt, :], src[:, kt, :],
                            start=(kt == 0), stop=(kt == 3),
                        )
                    nc.scalar.activation(
                        dst[:], ps[:], Ident, bias=bias[b][:, h:h + 1]
                    )
                QROT = pwork.tile([P, 4, S], bf16, tag="qrot")
                rope(QROT, QRAW, "qr")
                qk_tiles[h] = (qc, kc, QROT)

            def phase_b(h):
                qc, kc, QROT = qk_tiles.pop(h)
                # scores_T [k_pos, q_pos] -> E_T = exp(s/SCALE), bf16
                E = pwork.tile([P, 4, S], bf16, tag="E")
                for kp in range(4):
                    ps = pps.tile([P, 512], f32, tag="ps")
                    pieces = [(kc[:, kp * P:(kp + 1) * P], qc[:])]
                    pieces += [
                        (KROT[:, f, kp * P:(kp + 1) * P], QROT[:, f, :])
                        for f in range(4)
                    ]
                    for i, (lhsT, rhs) in enumerate(pieces):
                        nc.tensor.matmul(
                            ps[:], lhsT, rhs, start=(i == 0), stop=(i == 4)
                        )
                    nc.scalar.activation(
                        E[:, kp, :], ps[:], Exp, scale=1.0 / SCALE
                    )

                # o_raw^T [VHD, S] fm (unnormalized)
                ps_o = pps.tile([P, 512], f32, tag="ps")
                for kp in range(4):
                    nc.tensor.matmul(
                        ps_o[:],
                        VC[:, kp, h * VHD:(h + 1) * VHD],
                        E[:, kp, :],
                        start=(kp == 0),
                        stop=(kp == 3),
                    )
                nc.any.tensor_copy(ORAW[:, h, :], ps_o[:])

                # sums[h] = sum_k E_T; engines can't write at partition h, so
                # stage on partition 0 and DMA into the dram gather buffer
                ps_s = pps1.tile([1, 512], f32, tag="ps1")
                for kp in range(4):
                    nc.tensor.matmul(
                        ps_s[:], ones_col[:], E[:, kp, :],
                        start=(kp == 0), stop=(kp == 3),
                    )
                sums_tmp = pwork.tile([1, S], f32, tag="sumtmp")
                nc.scalar.activation(sums_tmp[:], ps_s[:], Ident)
                nc.sync.dma_start(sums_dram[h * S:(h + 1) * S][None], sums_tmp[:])

                # after each half of the heads: pipelined recip + normalize
                if h % 8 == 7:
                    hb = h // 8
                    sl = slice(hb * 8 * S, (hb + 1) * 8 * S)
                    SUMS2 = pwork.tile([P, 8 * S // P], f32, tag="sums2")
                    nc.sync.dma_start(
                        SUMS2[:], sums_dram[sl].rearrange("(p j) -> p j", p=P)
                    )
                    RECIP2 = pwork.tile([P, 8 * S // P], bf16, tag="recip2")
                    with nc.allow_low_precision(reason="softmax denom recip"):
                        nc.vector.reciprocal(RECIP2[:], SUMS2[:])
                    nc.sync.dma_start(
                        recip_dram[sl].rearrange("(p j) -> p j", p=P), RECIP2[:]
                    )
                    for hh in range(hb * 8, (hb + 1) * 8):
                        rb = pwork.tile([P, S], bf16, tag="rb")
                        nc.sync.dma_start(
                            rb[:],
                            recip_dram[hh * S:(hh + 1) * S][None]
                            .to_broadcast((P, S)),
                        )
                        nc.vector.tensor_tensor(
                            ORAW[:, hh, :], ORAW[:, hh, :], rb[:], MUL
                        )
                        nc.vector.tensor_scalar_add(
                            ORAW[:, hh, :], ORAW[:, hh, :], bias["buv"][:, hh:hh + 1]
                        )

            phase_a(0)
            for h in range(16):
                if h + 1 < 16:
                    phase_a(h + 1)
                phase_b(h)

            # ---- stage 5: fc ----
            for mt in range(16):
                WFCt = pbig.tile([P, 16, P], bf16, tag=f"w1_{mt % 12}")
                nc.sync.dma_start(WFCt[:], wfc_d[mt])
                ps = pps.tile([P, 512], f32, tag="ps")
                for kt in range(16):
                    nc.tensor.matmul(
                        ps[:], WFCt[:, kt, :], ORAW[:, kt, :],
                        start=(kt == 0), stop=(kt == 15),
                    )
                y = pwork.tile([P, 512], f32, tag="y")
                nc.scalar.activation(
                    y[:], ps[:], Ident, bias=bias["bfc"][:, mt:mt + 1]
                )
                nc.sync.dma_start(yt_d[mt * P:(mt + 1) * P, :], y[:])

    nc.compile()
    return nc, input_names


def prepare_in_maps(inputs):
    """Host-side prep: cast to bf16, de-interleave rope dims, tile layouts."""
    g = {k: np.asarray(v, dtype=np.float32) for k, v in inputs.items()}
    perm = np.concatenate([np.arange(0, R, 2), np.arange(1, R, 2)])

    def chunks16(w):  # [2048, M] -> 4 chunks [128, 4, M]
        kt = w.reshape(16, P, w.shape[1])
        return [
            np.ascontiguousarray(kt[4 * c:4 * c + 4].transpose(1, 0, 2)).astype(BF16)
            for c in range(4)
        ]

    def bcol(b, k):  # [k*128] -> [128, k] fp32
        return np.ascontiguousarray(b.reshape(k, P).T)

    wqr_p = g["Wqr"].reshape(DOWN, H, R)[:, :, perm]
    wkr_p = g["Wkr"][:, perm]
    bqr_p = g["bqr"].reshape(H, R)[:, perm]
    bkr_p = g["bkr"][perm]

    common = {}
    for c, (a, b_, d) in enumerate(
        zip(chunks16(g["Wdq"]), chunks16(g["Wdkv"]), chunks16(wkr_p))
    ):
        common[f"wdq{c}"] = a
        common[f"wdkv{c}"] = b_
        common[f"wkr{c}"] = d
    common["wuq"] = np.ascontiguousarray(
        g["Wuq"].reshape(4, P, H, VHD).transpose(2, 1, 0, 3)).astype(BF16)
    common["wuk"] = np.ascontiguousarray(
        g["Wuk"].reshape(4, P, H, VHD).transpose(2, 1, 0, 3)).astype(BF16)
    common["wuv"] = np.ascontiguousarray(
        g["Wuv"].reshape(4, P, UP).transpose(1, 0, 2)).astype(BF16)
    common["wqr"] = np.ascontiguousarray(
        wqr_p.transpose(1, 0, 2).reshape(H, 4, P, R).transpose(0, 2, 1, 3)
    ).astype(BF16)
    common["wfc"] = np.ascontiguousarray(
        g["Wfc"].reshape(16, P, 16, P).transpose(2, 1, 0, 3)).astype(BF16)

    sin_t, cos_t = _rope_tables_np()      # [512, 256]
    for nm, t in (("cosr", cos_t), ("sinr", sin_t)):
        common[nm] = np.ascontiguousarray(
            t.T.reshape(2, P, S).transpose(1, 0, 2)).astype(BF16)

    common["bdq"] = bcol(g["bdq"], 4)
    common["bdkv"] = bcol(g["bdkv"], 4)
    common["bkr"] = bcol(bkr_p, 4)
    common["buq"] = bcol(g["buq"], H)
    common["buk"] = bcol(g["buk"], H)
    common["buv"] = bcol(g["buv"], H)
    common["bqr"] = bcol(bqr_p.reshape(-1), 64)
    common["bfc"] = bcol(g["bfc"], 16)

    in_maps = []
    for b in range(B):
        m = dict(common)
        xtb = g["X"][b].T.astype(BF16)        # [2048, 512]
        kt = xtb.reshape(16, P, S)
        for c in range(4):
            m[f"xt{c}"] = np.ascontiguousarray(
                kt[4 * c:4 * c + 4].transpose(1, 0, 2))
        in_maps.append(m)
    return in_maps


def _get_program():
    if "nc" not in _CACHE:
        _CACHE["nc"], _CACHE["input_names"] = build_nc()
    return _CACHE["nc"], _CACHE["input_names"]


def kernel(**inputs) -> np.ndarray:
    from concourse.bass_utils import run_bass_kernel_spmd

    nc, _ = _get_program()
    in_maps = prepare_in_maps(inputs)
    res = run_bass_kernel_spmd(nc, in_maps, core_ids=list(range(B)))
    out = np.stack(
        [np.ascontiguousarray(res.results[b]["yt"].T) for b in range(B)]
    )
    return out.astype(np.float32)



# revision 4
# speedup vs baseline: 1.3244x; 1.3244x over previous
"""MLA (multi-head latent attention) Bass kernel for Trainium2, 8 NeuronCores.

Sharding: pure data-parallel over batch (B=8 -> one batch element per core).
Each core runs the full per-batch computation; no collectives.

v2 changes over the original baseline:
  - Inputs packed into 4 DRAM tensors per core (xt bf16, xt8 fp8, wb bf16
    blob, w8 fp8 blob) to cut per-dispatch host overhead.
  - All softmax-damped matmuls run in fp8e4m3 with DoubleRow perf mode
    (K=256 per matmul): the k_r / q_r projections, the q_c / k_c
    up-projections, and the rope part of the attention scores. Their
    quantization error only perturbs the softmax exponent (scale ~1/34),
    so the end-to-end error stays well under the gate.
  - Softmax denominators: E tiles pre-summed on GpSimd (idle engine), then
    a single M=1 ones-matmul per head instead of four.
  - Rope muls split DVE/GpSimd; kc-activation and sums-copy moved to DVE so
    no engine exceeds the PE per-head budget.
  - Stage-0 DMAs issued on both HWDGE queues (Sync + Scalar) to compress
    the start ramp.

Layouts (per core):
  - Activations are feature-major ("fm"): [feature_partitions, tokens].
  - Scores are computed transposed (scores_T[k_pos, q_pos]); softmax skips
    max-subtraction (|s/scale| < ~3); exp fused with 1/scale on ScalarE.
  - RoPE pairs de-interleaved on the host by permuting Wqr/Wkr columns.
  - Normalization deferred: o_raw accumulates unnormalized; reciprocals of
    8 heads' sums at a time via one compact DVE op, broadcast by DMA, fused
    with the +b_uv bias.
"""

import sys

import numpy as np
import ml_dtypes

for _p in ("/opt/trn_rl_repo",):
    if _p not in sys.path:
        sys.path.append(_p)

B, S, D, H = 8, 512, 2048, 16
DOWN, UP, R = 512, 2048, 512
VHD = UP // H          # 128
HD = D // H            # 128
SCALE = float(HD**0.5 + R**0.5)
P = 128
BF16 = ml_dtypes.bfloat16
FP8 = ml_dtypes.float8_e4m3

_CACHE = {}

# --- wb (bf16) blob column offsets, [128, WB_COLS] ---
WB_WDQ = 0                      # [128, 16, 512] kt-major
WB_WDKV = WB_WDQ + 16 * 512
WB_COS = WB_WDKV + 16 * 512     # [128, 2, 512]
WB_SIN = WB_COS + 2 * 512
WB_BIAS = WB_SIN + 2 * 512      # [128, 140]
WB_WUV = WB_BIAS + 140          # [128, 4, 2048]
WB_WFC = WB_WUV + 4 * 2048      # [128, 16, 16, 128] mt-major
WB_COLS = WB_WFC + 16 * 16 * 128

# bias columns within the [128, 140] bias tile
BC_BDQ, BC_BDKV, BC_BKR = 0, 4, 8
BC_BUQ, BC_BUK, BC_BUV = 12, 28, 44
BC_BQR, BC_BFC = 60, 124

# --- w8 (fp8) blob column offsets, [128, W8_COLS] ---
W8_WKR = 0                      # [128, 16, 512] kt-major
W8_HEAD = W8_WKR + 16 * 512     # per head: [128, 4, 768] = wqr|wuq|wuk
W8_HEAD_COLS = 4 * 768
W8_COLS = W8_HEAD + H * W8_HEAD_COLS


def _rope_tables_np():
    pos = np.arange(R, dtype=np.float32)
    div = np.exp(np.arange(0, R, 2, dtype=np.float32) * (-np.log(10000.0) / R))
    theta = np.outer(pos, div)          # [512, 256]
    return np.sin(theta), np.cos(theta)


def build_nc():
    """Build + compile the per-core Bass program."""
    import concourse.mybir as mybir
    import concourse.tile as tile
    from concourse import bacc

    f32 = mybir.dt.float32
    bf16 = mybir.dt.bfloat16
    fp8 = mybir.dt.float8e4
    Ident = mybir.ActivationFunctionType.Identity
    Exp = mybir.ActivationFunctionType.Exp
    MUL = mybir.AluOpType.mult
    ADD = mybir.AluOpType.add
    SUB = mybir.AluOpType.subtract
    DR = mybir.MatmulPerfMode.DoubleRow

    nc = bacc.Bacc(
        "TRN2",
        target_bir_lowering=False,
        debug=False,
        enable_asserts=False,
        num_devices=8,
    )

    xt_d = nc.dram_tensor("xt", [P, 16, S], bf16, kind="ExternalInput").ap()
    xt8_d = nc.dram_tensor("xt8", [P, 16, S], fp8, kind="ExternalInput").ap()
    wb_d = nc.dram_tensor("wb", [P, WB_COLS], bf16, kind="ExternalInput").ap()
    w8_d = nc.dram_tensor("w8", [P, W8_COLS], fp8, kind="ExternalInput").ap()
    yt_d = nc.dram_tensor("yt", [D, S], f32, kind="ExternalOutput").ap()

    def wb3(off, a, b):
        return wb_d[:, off:off + a * b].rearrange("p (a b) -> p a b", a=a)

    def w83(off, a, b):
        return w8_d[:, off:off + a * b].rearrange("p (a b) -> p a b", a=a)

    with tile.TileContext(nc) as tc:
        with (
            tc.tile_pool(name="pconst", bufs=1) as pconst,
            tc.tile_pool(name="pbig", bufs=1) as pbig,
            tc.tile_pool(name="pwork", bufs=2) as pwork,
            tc.tile_pool(name="pps", bufs=7, space="PSUM") as pps,
            tc.tile_pool(name="pps1", bufs=1, space="PSUM") as pps1,
            tc.tile_pool(name="pdram", bufs=1, space="DRAM") as pdram,
        ):
            lp = nc.allow_low_precision(reason="fp8 softmax-damped path")
            lp.__enter__()

            # ---- stage-0 DMAs; first matmul needs xt kt0 + wdq kt0 only ----
            XT = pbig.tile([P, 16, S], bf16, tag="xt")
            nc.sync.dma_start(XT[:, 0:1, :], xt_d[:, 0:1, :])
            WDQ = pbig.tile([P, 16, S], bf16, tag="wdq")
            wdq_ap = wb3(WB_WDQ, 16, 512)
            nc.scalar.dma_start(WDQ[:, 0:1, :], wdq_ap[:, 0:1, :])
            nc.sync.dma_start(XT[:, 1:4, :], xt_d[:, 1:4, :])
            nc.scalar.dma_start(WDQ[:, 1:4, :], wdq_ap[:, 1:4, :])
            for c in range(1, 4):
                nc.sync.dma_start(XT[:, 4 * c:4 * c + 4, :], xt_d[:, 4 * c:4 * c + 4, :])
                nc.scalar.dma_start(
                    WDQ[:, 4 * c:4 * c + 4, :], wdq_ap[:, 4 * c:4 * c + 4, :]
                )
            WDKV = pbig.tile([P, 16, S], bf16, tag="wdkv")
            wdkv_ap = wb3(WB_WDKV, 16, 512)
            for c in range(4):
                nc.scalar.dma_start(
                    WDKV[:, 4 * c:4 * c + 4, :], wdkv_ap[:, 4 * c:4 * c + 4, :]
                )
            BIASB = pconst.tile([P, 140], bf16, tag="biasb")
            nc.scalar.dma_start(BIASB[:], wb_d[:, WB_BIAS:WB_BIAS + 140])
            BIAS = pconst.tile([P, 140], f32, tag="bias")
            nc.vector.tensor_copy(BIAS[:], BIASB[:])

            def bcol(base, j):
                return BIAS[:, base + j:base + j + 1]

            XT8 = pbig.tile([P, 16, S], fp8, tag="xt8")
            WKR8 = pbig.tile([P, 16, S], fp8, tag="wkr8")
            wkr_ap = w83(W8_WKR, 16, 512)
            for c in range(2):
                nc.sync.dma_start(XT8[:, 8 * c:8 * c + 8, :], xt8_d[:, 8 * c:8 * c + 8, :])
                nc.sync.dma_start(WKR8[:, 8 * c:8 * c + 8, :], wkr_ap[:, 8 * c:8 * c + 8, :])
            cos_t = pconst.tile([P, 2, S], bf16, tag="cos_t")
            sin_t = pconst.tile([P, 2, S], bf16, tag="sin_t")
            nc.sync.dma_start(cos_t[:], wb3(WB_COS, 2, 512))
            nc.sync.dma_start(sin_t[:], wb3(WB_SIN, 2, 512))
            WUV = pbig.tile([P, 4, UP], bf16, tag="wuv")
            wuv_ap = wb3(WB_WUV, 4, 2048)
            nc.scalar.dma_start(WUV[:, 0:2, :], wuv_ap[:, 0:2, :])
            nc.scalar.dma_start(WUV[:, 2:4, :], wuv_ap[:, 2:4, :])
            ones_col = pconst.tile([P, 1], bf16, tag="ones_col")
            nc.vector.memset(ones_col[:], 1.0)

            # persistent activations
            CQ8 = pbig.tile([P, 4, S], fp8, tag="cq8")
            CKV = pbig.tile([P, 4, S], bf16, tag="ckv")
            CKV8 = pbig.tile([P, 4, S], fp8, tag="ckv8")
            KROT8 = pbig.tile([P, 4, S], fp8, tag="krot8")
            VC = pbig.tile([P, 4, UP], bf16, tag="vc")
            ORAW = pbig.tile([P, H, S], bf16, tag="oraw")
            sums_dram = pdram.tile([H * S], f32)
            recip_dram = pdram.tile([H * S], bf16)

            # ---- stage 1: c_q (fp8 out), c_kv (bf16 + fp8), k_r (DR fp8) ----
            for dst, W, b, d8 in (
                (CQ8, WDQ, BC_BDQ, None), (CKV, WDKV, BC_BDKV, CKV8),
            ):
                pss = [
                    pps.tile([P, 512], f32, tag="ps", name=f"ps_s1_{i}")
                    for i in range(4)
                ]
                for kt in range(16):
                    for mt in range(4):
                        nc.tensor.matmul(
                            pss[mt][:],
                            W[:, kt, mt * P:(mt + 1) * P],
                            XT[:, kt, :],
                            start=(kt == 0),
                            stop=(kt == 15),
                        )
                for mt in range(4):
                    nc.scalar.activation(
                        dst[:, mt, :], pss[mt][:], Ident, bias=bcol(b, mt)
                    )
                if d8 is not None:
                    nc.vector.tensor_copy(
                        d8[:].rearrange("p a b -> p (a b)"),
                        dst[:].rearrange("p a b -> p (a b)"),
                    )

            # k_raw via fp8 DoubleRow, then rope -> KROT8
            pss = [
                pps.tile([P, 512], f32, tag="ps", name=f"ps_kr_{i}")
                for i in range(4)
            ]
            for i in range(8):
                for mt in range(4):
                    nc.tensor.matmul(
                        pss[mt][:],
                        WKR8[:, 2 * i:2 * i + 2, mt * P:(mt + 1) * P],
                        XT8[:, 2 * i:2 * i + 2, :],
                        start=(i == 0),
                        stop=(i == 7),
                        perf_mode=DR,
                    )
            KRAW = pwork.tile([P, 4, S], bf16, tag="kraw")
            for mt in range(4):
                nc.scalar.activation(
                    KRAW[:, mt, :], pss[mt][:], Ident, bias=bcol(BC_BKR, mt),
                    scale=1.0 / 32.0,
                )
            # rope k (one-time; all on DVE)
            for i in range(2):
                x1 = KRAW[:, i, :]
                x2 = KRAW[:, 2 + i, :]
                c_, s_ = cos_t[:, i, :], sin_t[:, i, :]
                t1 = pwork.tile([P, S], bf16, tag="kra")
                t2 = pwork.tile([P, S], bf16, tag="krb")
                nc.vector.tensor_tensor(t1[:], x1, c_, MUL)
                nc.vector.tensor_tensor(t2[:], x2, s_, MUL)
                nc.vector.tensor_tensor(KROT8[:, i, :], t1[:], t2[:], SUB)
                t3 = pwork.tile([P, S], bf16, tag="kra")
                t4 = pwork.tile([P, S], bf16, tag="krb")
                nc.vector.tensor_tensor(t3[:], x2, c_, MUL)
                nc.vector.tensor_tensor(t4[:], x1, s_, MUL)
                nc.vector.tensor_tensor(KROT8[:, 2 + i, :], t3[:], t4[:], ADD)

            # ---- stage 2: v_c token-major (no bias; folded into o-norm) ----
            for tt in range(4):
                for hg in range(4):
                    ps = pps.tile([P, 512], f32, tag="ps")
                    for kt in range(4):
                        nc.tensor.matmul(
                            ps[:],
                            CKV[:, kt, tt * P:(tt + 1) * P],
                            WUV[:, kt, hg * 512:(hg + 1) * 512],
                            start=(kt == 0),
                            stop=(kt == 3),
                        )
                    nc.any.tensor_copy(VC[:, tt, hg * 512:(hg + 1) * 512], ps[:])

            # ---- stage 3: per-head attention, software-pipelined ----
            qk_tiles = {}

            def phase_a(h):
                W8h = pwork.tile([P, 4, 768], fp8, tag="w8h")
                nc.sync.dma_start(
                    W8h[:], w83(W8_HEAD + h * W8_HEAD_COLS, 4, 768)
                )

                # q_r raw projection [R, S] fm via DoubleRow
                psq = [
                    pps.tile([P, 512], f32, tag="ps", name=f"ps_qr{h}_{i}")
                    for i in range(4)
                ]
                for mt in range(4):
                    for i in range(2):
                        nc.tensor.matmul(
                            psq[mt][:],
                            W8h[:, 2 * i:2 * i + 2, mt * P:(mt + 1) * P],
                            CQ8[:, 2 * i:2 * i + 2, :],
                            start=(i == 0),
                            stop=(i == 1),
                            perf_mode=DR,
                        )
                QRAW = pwork.tile([P, 4, S], bf16, tag="qraw")
                for mt in range(4):
                    nc.scalar.activation(
                        QRAW[:, mt, :], psq[mt][:], Ident,
                        bias=bcol(BC_BQR, h * 4 + mt), scale=1.0 / 32.0,
                    )
                # q_c, k_c [VHD, S] fm via DoubleRow, fp8 out
                qc = pwork.tile([P, S], fp8, tag="qc")
                kc = pwork.tile([P, S], fp8, tag="kc")
                for dst, coff, b, src, eng in (
                    (qc, 512, BC_BUQ, CQ8, "scalar"),
                    (kc, 640, BC_BUK, CKV8, "vector"),
                ):
                    ps = pps.tile([P, 512], f32, tag="ps")
                    for i in range(2):
                        nc.tensor.matmul(
                            ps[:],
                            W8h[:, 2 * i:2 * i + 2, coff:coff + P],
                            src[:, 2 * i:2 * i + 2, :],
                            start=(i == 0),
                            stop=(i == 1),
                            perf_mode=DR,
                        )
                    if eng == "scalar":
                        nc.scalar.activation(
                            dst[:], ps[:], Ident, bias=bcol(b, h),
                            scale=1.0 / 32.0,
                        )
                    else:
                        nc.vector.tensor_scalar(
                            dst[:], ps[:], 1.0 / 32.0, bcol(b, h), MUL, ADD
                        )
                # rope q: t1/t2 + combines on DVE, t3/t4 on GpSimd
                QROT8 = pwork.tile([P, 4, S], fp8, tag="qrot8")
                for i in range(2):
                    x1 = QRAW[:, i, :]
                    x2 = QRAW[:, 2 + i, :]
                    c_, s_ = cos_t[:, i, :], sin_t[:, i, :]
                    t1 = pwork.tile([P, S], bf16, tag="qra")
                    t2 = pwork.tile([P, S], bf16, tag="qrb")
                    nc.vector.tensor_tensor(t1[:], x1, c_, MUL)
                    nc.vector.tensor_tensor(t2[:], x2, s_, MUL)
                    nc.vector.tensor_tensor(QROT8[:, i, :], t1[:], t2[:], SUB)
                    t3 = pwork.tile([P, S], bf16, tag="qrc")
                    t4 = pwork.tile([P, S], bf16, tag="qrd")
                    nc.gpsimd.tensor_tensor(t3[:], x2, c_, MUL)
                    nc.gpsimd.tensor_tensor(t4[:], x1, s_, MUL)
                    nc.vector.tensor_tensor(QROT8[:, 2 + i, :], t3[:], t4[:], ADD)
                qk_tiles[h] = (qc, kc, QROT8)

            def phase_b(h):
                qc, kc, QROT8 = qk_tiles.pop(h)
                # scores_T -> E = exp(s/SCALE): content fp8 + rope DoubleRow
                E = pwork.tile([P, 4, S], bf16, tag="E")
                for kp in range(4):
                    ps = pps.tile([P, 512], f32, tag="ps")
                    nc.tensor.matmul(
                        ps[:], kc[:, kp * P:(kp + 1) * P], qc[:],
                        start=True, stop=False,
                    )
                    for i in range(2):
                        nc.tensor.matmul(
                            ps[:],
                            KROT8[:, 2 * i:2 * i + 2, kp * P:(kp + 1) * P],
                            QROT8[:, 2 * i:2 * i + 2, :],
                            start=False,
                            stop=(i == 1),
                            perf_mode=DR,
                        )
                    nc.scalar.activation(
                        E[:, kp, :], ps[:], Exp, scale=1.0 / SCALE
                    )

                # o_raw^T [VHD, S] fm (unnormalized)
                ps_o = pps.tile([P, 512], f32, tag="ps")
                for kp in range(4):
                    nc.tensor.matmul(
                        ps_o[:],
                        VC[:, kp, h * VHD:(h + 1) * VHD],
                        E[:, kp, :],
                        start=(kp == 0),
                        stop=(kp == 3),
                    )
                nc.any.tensor_copy(ORAW[:, h, :], ps_o[:])

                # sums[h]: pre-add E tiles on GpSimd, one M=1 matmul
                ea = pwork.tile([P, S], bf16, tag="ea")
                eb = pwork.tile([P, S], bf16, tag="eb")
                nc.gpsimd.tensor_tensor(ea[:], E[:, 0, :], E[:, 1, :], ADD)
                nc.gpsimd.tensor_tensor(eb[:], E[:, 2, :], E[:, 3, :], ADD)
                esum = pwork.tile([P, S], bf16, tag="esum")
                nc.gpsimd.tensor_tensor(esum[:], ea[:], eb[:], ADD)
                ps_s = pps1.tile([1, 512], f32, tag="ps1")
                nc.tensor.matmul(ps_s[:], ones_col[:], esum[:], start=True, stop=True)
                sums_tmp = pwork.tile([1, S], f32, tag="sumtmp")
                nc.vector.tensor_copy(sums_tmp[:], ps_s[:])
                nc.sync.dma_start(sums_dram[h * S:(h + 1) * S][None], sums_tmp[:])

                # after each half of the heads: pipelined recip + normalize
                if h % 8 == 7:
                    hb = h // 8
                    sl = slice(hb * 8 * S, (hb + 1) * 8 * S)
                    SUMS2 = pwork.tile([P, 8 * S // P], f32, tag="sums2")
                    nc.sync.dma_start(
                        SUMS2[:], sums_dram[sl].rearrange("(p j) -> p j", p=P)
                    )
                    RECIP2 = pwork.tile([P, 8 * S // P], bf16, tag="recip2")
                    nc.vector.reciprocal(RECIP2[:], SUMS2[:])
                    nc.sync.dma_start(
                        recip_dram[sl].rearrange("(p j) -> p j", p=P), RECIP2[:]
                    )
                    for hh in range(hb * 8, (hb + 1) * 8):
                        rb = pwork.tile([P, S], bf16, tag="rb")
                        nc.sync.dma_start(
                            rb[:],
                            recip_dram[hh * S:(hh + 1) * S][None]
                            .to_broadcast((P, S)),
                        )
                        nc.vector.tensor_tensor(
                            ORAW[:, hh, :], ORAW[:, hh, :], rb[:], MUL
                        )
                        nc.vector.tensor_scalar_add(
                            ORAW[:, hh, :], ORAW[:, hh, :], bcol(BC_BUV, hh)
                        )

            phase_a(0)
            for h in range(16):
                if h + 1 < 16:
                    phase_a(h + 1)
                phase_b(h)

            # ---- stage 5: fc ----
            wfc_ap = wb_d[:, WB_WFC:].rearrange(
                "p (m k c) -> p m k c", m=16, k=16
            )
            for mt in range(16):
                WFCt = pbig.tile([P, 16, P], bf16, tag=f"wfc_{mt % 3}")
                nc.sync.dma_start(WFCt[:], wfc_ap[:, mt])
                ps = pps.tile([P, 512], f32, tag="ps")
                for kt in range(16):
                    nc.tensor.matmul(
                        ps[:], WFCt[:, kt, :], ORAW[:, kt, :],
                        start=(kt == 0), stop=(kt == 15),
                    )
                y = pwork.tile([P, 512], f32, tag="y")
                nc.scalar.activation(
                    y[:], ps[:], Ident, bias=bcol(BC_BFC, mt)
                )
                nc.sync.dma_start(yt_d[mt * P:(mt + 1) * P, :], y[:])

            lp.__exit__(None, None, None)

    nc.compile()
    return nc, ["xt", "xt8", "wb", "w8"]


def prepare_in_maps(inputs):
    """Host-side prep: cast, de-interleave rope dims, pack blobs."""
    g = {k: np.asarray(v, dtype=np.float32) for k, v in inputs.items()}
    perm = np.concatenate([np.arange(0, R, 2), np.arange(1, R, 2)])

    def ktmajor(w):  # [2048, M] -> [128, 16*M], kt-major cols
        kt = w.reshape(16, P, w.shape[1])
        return np.ascontiguousarray(kt.transpose(1, 0, 2)).reshape(P, -1)

    def bcol(b, k):  # [k*128] -> [128, k]
        return np.ascontiguousarray(b.reshape(k, P).T)

    wqr_p = g["Wqr"].reshape(DOWN, H, R)[:, :, perm]
    wkr_p = g["Wkr"][:, perm]
    bqr_p = g["bqr"].reshape(H, R)[:, perm]
    bkr_p = g["bkr"][perm]

    # ---- wb bf16 blob ----
    wb = np.empty((P, WB_COLS), dtype=BF16)
    wb[:, WB_WDQ:WB_WDQ + 16 * 512] = ktmajor(g["Wdq"]).astype(BF16)
    wb[:, WB_WDKV:WB_WDKV + 16 * 512] = ktmajor(g["Wdkv"]).astype(BF16)
    sin_t, cos_t = _rope_tables_np()      # [512, 256]
    for off, t in ((WB_COS, cos_t), (WB_SIN, sin_t)):
        wb[:, off:off + 1024] = np.ascontiguousarray(
            t.T.reshape(2, P, S).transpose(1, 0, 2)).reshape(P, -1).astype(BF16)
    bias = np.zeros((P, 140), dtype=np.float32)
    bias[:, BC_BDQ:BC_BDQ + 4] = bcol(g["bdq"], 4)
    bias[:, BC_BDKV:BC_BDKV + 4] = bcol(g["bdkv"], 4)
    bias[:, BC_BKR:BC_BKR + 4] = bcol(bkr_p, 4)
    bias[:, BC_BUQ:BC_BUQ + H] = bcol(g["buq"], H)
    bias[:, BC_BUK:BC_BUK + H] = bcol(g["buk"], H)
    bias[:, BC_BUV:BC_BUV + H] = bcol(g["buv"], H)
    bias[:, BC_BQR:BC_BQR + 64] = bcol(bqr_p.reshape(-1), 64)
    bias[:, BC_BFC:BC_BFC + 16] = bcol(g["bfc"], 16)
    wb[:, WB_BIAS:WB_BIAS + 140] = bias.astype(BF16)
    wb[:, WB_WUV:WB_WUV + 4 * 2048] = np.ascontiguousarray(
        g["Wuv"].reshape(4, P, UP).transpose(1, 0, 2)).reshape(P, -1).astype(BF16)
    wb[:, WB_WFC:] = np.ascontiguousarray(
        g["Wfc"].reshape(16, P, 16, P).transpose(1, 2, 0, 3)
    ).reshape(P, -1).astype(BF16)

    # ---- w8 fp8 blob ----
    w8 = np.empty((P, W8_COLS), dtype=FP8)
    w8[:, W8_WKR:W8_WKR + 16 * 512] = ktmajor(wkr_p * 32.0).astype(FP8)
    wqr_h = np.ascontiguousarray(
        wqr_p.transpose(1, 0, 2).reshape(H, 4, P, R).transpose(0, 2, 1, 3) * 32.0
    ).astype(FP8)                                    # [H, 128, 4, 512]
    wuq_h = np.ascontiguousarray(
        g["Wuq"].reshape(4, P, H, VHD).transpose(2, 1, 0, 3) * 32.0).astype(FP8)
    wuk_h = np.ascontiguousarray(
        g["Wuk"].reshape(4, P, H, VHD).transpose(2, 1, 0, 3) * 32.0).astype(FP8)
    for h in range(H):
        blk = np.concatenate(
            [wqr_h[h], wuq_h[h], wuk_h[h]], axis=2)   # [128, 4, 768]
        off = W8_HEAD + h * W8_HEAD_COLS
        w8[:, off:off + W8_HEAD_COLS] = blk.reshape(P, -1)

    in_maps = []
    for b in range(B):
        xtb = np.ascontiguousarray(g["X"][b].T).reshape(16, P, S).transpose(1, 0, 2)
        xtb = np.ascontiguousarray(xtb)
        in_maps.append({
            "xt": xtb.astype(BF16),
            "xt8": xtb.astype(FP8),
            "wb": wb,
            "w8": w8,
        })
    return in_maps


def _get_program():
    if "nc" not in _CACHE:
        _CACHE["nc"], _CACHE["input_names"] = build_nc()
    return _CACHE["nc"], _CACHE["input_names"]


def kernel(**inputs) -> np.ndarray:
    from concourse.bass_utils import run_bass_kernel_spmd

    nc, _ = _get_program()
    in_maps = prepare_in_maps(inputs)
    res = run_bass_kernel_spmd(nc, in_maps, core_ids=list(range(B)))
    out = np.stack(
        [np.ascontiguousarray(res.results[b]["yt"].T) for b in range(B)]
    )
    return out.astype(np.float32)
